# revision 14
# baseline (speedup 1.0000x reference)
# Multi-head causal self-attention (B=2, S=2048, D=1024, H=16, Dh=64) on 8
# Trainium2 NeuronCores.
#
# Sharding: core i -> (batch b = i // 4, head-group g = i % 4). Each core
# computes attention for its batch's 4 heads (feature columns 256g:256g+256 of
# the QKV projections, rows 256g:256g+256 of Wo) and produces a partial
# out-projection [S, D]. Host sums the 4 partials per batch and adds bo.
#
# All matmul operands are bf16 (fp32 PSUM accumulation), ~1 cycle/row on the
# PE vs 1.5 for f32r; ~4e-3 rel error, well under the 2e-2 gate.
#
# All DMAs are fully contiguous: the host pre-shapes every tensor into its
# SBUF layout ([partition, ...] with contiguous free dims) so descriptor
# generation is trivial and packets are large (8-32KB vs 512B-1KB for the
# rearranged gathers this replaced; those delayed first compute by ~2us).
#
# Per-core dataflow:
#   1. tiny exp() at t=0 preloads the ACT table set (~2.7us) during DMA wait,
#      off the first-attention critical path.
#   2. QT = Wq_s^T xT + bq [256, S]: PSUM->SBUF move + bias ride one DVE
#      tensor_scalar_add (keeps ScalarE free for exp, which paces the
#      attention phase). KT [256, S]: K's bias is dropped -- (q+bq).(k+bk)
#      differs from (q+bq).k by a per-query-row constant, which softmax
#      cancels. Head pair p keeps head 2p on partitions 0:64, head 2p+1 on
#      64:128. V = xT^T Wv_s + bv [S, 256], stored augmented with a ones
#      column per head ([V_h | 1]) so the attention matmul also accumulates
#      the softmax denominator.
#   3. per (head-pair, k-block j): two CONCURRENT K=64 row-tiled matmuls
#      (tile_position (0,0) / (64,0)) produce both heads' scores S^T [k, q]
#      into one 2-bank PSUM tile; ONE ScalarE exp covers both heads
#      (ScalarE is the exp bottleneck: cost ~ (N+352)/1.2 ns, so batching
#      heads halves the fixed overhead + semaphore traffic). Scores are
#      pre-scaled by 1/sqrt(Dh) via host-side Wq scaling; magnitudes are
#      small enough that max-subtraction is unnecessary. Causality = skip
#      k>q blocks + ONE triangular mask multiply covering both heads on
#      diagonal blocks. [ctx^T; denom] += [V_h | 1]^T E per head.
#   4. normalize: recip(denom) on DVE, partition-broadcast via two K=1
#      column-tiled matmuls, multiply -> ctxT bf16.
#   5. out_partial = ctxT^T Wo_s per 128-row block, DVE copy, DMA out.
#
# A few dummy matmuls at t=0 keep the PE busy while the first DMAs land so
# the HAM clock-gate un-throttles (4/8 -> 8/8) before real work starts.

import numpy as np
import ml_dtypes

import concourse.bass as bass
import concourse.mybir as mybir
import concourse.tile as tile
from concourse import bacc
from concourse.bass_utils import run_bass_kernel_spmd
from concourse.masks import make_upper_triangular

F32 = mybir.dt.float32
BF16 = mybir.dt.bfloat16

B, S, D = 2, 2048, 1024
H, DH = 16, 64
NCORES = 8
GROUPS = 4               # head-groups (tensor parallel)
HG = H // GROUPS         # 4 heads per group
NPAIR = HG // 2          # 2 head-pairs per group
FEAT = HG * DH           # 256 features per group
SCALE = 1.0 / 8.0        # 1/sqrt(DH), folded into Wq/bq on host

CHUNK = 512              # seq chunk (PSUM bank = 512 fp32)
NSUB = CHUNK // 128      # 4 seq subtiles per chunk
NCHUNK = S // CHUNK      # 4
KD = D // 128            # 8 k-tiles over D
MT = FEAT // 128         # 2 feature M-tiles per group (m-tile == head-pair)


def _emit(tc):
    nc = tc.nc
    # host pre-shapes everything into SBUF layout -> contiguous DMAs
    # few, large, contiguous DMAs: wq alone (its arrival gates the first
    # matmul), everything else fused -- each extra DMA adds a hardware queue
    # whose teardown costs ~150ns/engine in the end-of-kernel semaphore storm
    xt_d = nc.dram_tensor("xt", [128, NCHUNK, KD, CHUNK], BF16,
                          kind="ExternalInput").ap()
    wq = nc.dram_tensor("wq", [128, KD, MT, 128], BF16, kind="ExternalInput").ap()
    # wk | wv | wo back to back (all [128, 2048] bf16 views)
    wr = nc.dram_tensor("wr", [128, 3, 2048], BF16, kind="ExternalInput").ap()
    # bq (MT) | bv (HG*DH) f32
    br = nc.dram_tensor("br", [128, MT + FEAT], F32, kind="ExternalInput").ap()
    out = nc.dram_tensor("out", [S, D], BF16, kind="ExternalOutput").ap()

    consts = tc.alloc_tile_pool(name="consts", bufs=1)
    weights = tc.alloc_tile_pool(name="weights", bufs=1)
    persist = tc.alloc_tile_pool(name="persist", bufs=1)
    qt_pool = tc.alloc_tile_pool(name="qt", bufs=2)
    et_pool = tc.alloc_tile_pool(name="et", bufs=8)
    rc_pool = tc.alloc_tile_pool(name="rc", bufs=2)
    ob_pool = tc.alloc_tile_pool(name="ob", bufs=2)
    work_ps = tc.alloc_tile_pool(name="work_ps", bufs=2, space="PSUM")
    sp_ps = tc.alloc_tile_pool(name="sp_ps", bufs=2, space="PSUM")
    cx_ps = tc.alloc_tile_pool(name="cx_ps", bufs=1, space="PSUM")

    # ---- preload the ACT exp table set while DMAs are in flight
    dum = consts.tile([1, 2], F32)
    nc.gpsimd.memset(dum, 0.0)
    nc.scalar.activation(dum[0:1, 1:2], dum[0:1, 0:1],
                         mybir.ActivationFunctionType.Exp)

    # ---- x^T chunk 0 first so projections can start ASAP
    xtall = persist.tile([128, NCHUNK, KD, CHUNK], BF16)
    nc.sync.dma_start(xtall[:, 0], xt_d[:, 0])

    # ---- weights
    wq_sb = weights.tile([128, KD, MT, 128], BF16)
    nc.sync.dma_start(wq_sb, wq)
    wr_sb = weights.tile([128, 3, 2048], BF16)
    nc.sync.dma_start(wr_sb, wr)
    br_sb = weights.tile([128, MT + FEAT], F32)
    nc.sync.dma_start(br_sb, br)
    wk_sb = wr_sb[:, 0, :].rearrange("p (k m f) -> p k m f", k=KD, m=MT)
    wv_sb = wr_sb[:, 1, :].rearrange("p (k f) -> p k f", k=KD)
    wo_sb = wr_sb[:, 2, :].rearrange("p (m d) -> p m d", m=MT)
    bqt = br_sb[:, 0:MT]
    bvb = br_sb[:, MT:].rearrange("p (h f) -> p h f", h=HG)

    # rest of x^T: chunk 1 alone (needed ~2 chunks in), chunks 2+3 fused
    nc.sync.dma_start(xtall[:, 1], xt_d[:, 1])
    nc.sync.dma_start(xtall[:, 2:4], xt_d[:, 2:4])

    # ---- constants
    onesf = consts.tile([128, 64], F32)
    nc.vector.memset(onesf, 1.0)
    # tri[k, q] = 1 if q >= k else 0; two copies side by side so ONE DVE op
    # masks both heads of a pair
    tri = consts.tile([128, 128], BF16)
    make_upper_triangular(nc, tri, val=1.0, diag=True)
    tri2 = consts.tile([128, 2, 128], BF16)
    nc.vector.tensor_copy(tri2[:, 0, :], tri)
    nc.vector.tensor_copy(tri2[:, 1, :], tri)
    wrm = consts.tile([128, CHUNK], BF16)
    nc.gpsimd.memset(wrm, 0.0)
    ones64 = consts.tile([1, 64], BF16)
    nc.vector.memset(ones64, 1.0)

    # ---- HAM warmup: PE busy while the first DMAs land (un-throttles the
    # clock gate to 8/8 before real work arrives). Sized to end just as the
    # first weights/x slices arrive (~13.7us; DMA-BW-bound) -- ending early
    # lets the HAM MID window re-throttle and the first chunk runs cold.
    for _ in range(26):
        wp = work_ps.tile([128, CHUNK], F32, tag="w", name="wp")
        nc.tensor.matmul(wp[:, 0:256], wrm[:, 0:128], wrm[:, 0:256],
                         start=True, stop=True)

    # ---- persistent activations
    # K^T pair-packed: pair p = heads (2p, 2p+1) on partitions 0:64 / 64:128
    kt2 = persist.tile([128, NPAIR, S], BF16)
    vaug = persist.tile([128, S // 128, HG, DH + 1], BF16)  # [V_h | 1] per head
    ctxT = persist.tile([128, MT, S], BF16)   # normalized ctx^T
    obh = persist.tile([128, NSUB, D], F32)   # final chunk's m=0 out partial
    nc.vector.tensor_copy(vaug[:, :, :, DH],
                          onesf.rearrange("p (a b) -> p a b", a=S // 128))

    def proj_q(c):
        qt = qt_pool.tile([128, MT, CHUNK], BF16, name="qt")
        for m in range(MT):
            ps = work_ps.tile([128, CHUNK], F32, tag="w", name="ps")
            for k in range(KD):
                nc.tensor.matmul(ps, wq_sb[:, k, m, :], xtall[:, c, k, :],
                                 start=(k == 0), stop=(k == KD - 1))
            nc.vector.tensor_scalar_add(qt[:, m, :], ps, bqt[:, m:m + 1])
        return qt

    def proj_k(c):
        cs = c * CHUNK
        for m in range(MT):
            ps = work_ps.tile([128, CHUNK], F32, tag="w", name="ps")
            for k in range(KD):
                nc.tensor.matmul(ps, wk_sb[:, k, m, :], xtall[:, c, k, :],
                                 start=(k == 0), stop=(k == KD - 1))
            nc.vector.tensor_copy(kt2[:, m, cs:cs + CHUNK], ps)

    def proj_v(c):
        for t in range(NSUB):
            gt = c * NSUB + t
            ps = work_ps.tile([128, CHUNK], F32, tag="w", name="ps")
            for k in range(KD):
                nc.tensor.matmul(ps[:, 0:FEAT],
                                 xtall[:, c, k, t * 128:(t + 1) * 128],
                                 wv_sb[:, k, :],
                                 start=(k == 0), stop=(k == KD - 1))
            nc.vector.tensor_add(
                vaug[:, gt, :, 0:DH],
                ps[:, 0:FEAT].rearrange("p (h f) -> p h f", h=HG), bvb)

    def attn(c, p, qt, cxa, cxb, j0, j1, first, last):
        # Two emission phases: all scores+exp(+mask) first, then all ctx
        # accumulations. The PE queue is in-order, so a ctx matmul stalled on
        # the previous pair's normalize chain (cx bank WAR) must sit BEHIND
        # this call's scores -- otherwise it head-of-line-blocks them and
        # starves ScalarE, the attention-phase pacer. et_pool provides the
        # scores->ctx elasticity.
        cs = c * CHUNK
        ets = []
        for j in range(j0, j1):
            lv = max(0, 128 * j - cs)   # first valid q (chunk-local)
            nq = CHUNK - lv
            sp = sp_ps.tile([128, 2, CHUNK], F32, tag="sp", name="sp")
            nc.tensor.matmul(sp[:, 0, 0:nq],
                             kt2[0:64, p, 128 * j:128 * (j + 1)],
                             qt[0:64, p, lv:CHUNK], start=True, stop=True)
            nc.tensor.matmul(sp[:, 1, 0:nq],
                             kt2[64:128, p, 128 * j:128 * (j + 1)],
                             qt[64:128, p, lv:CHUNK], start=True, stop=True)
            et = et_pool.tile([128, 2, CHUNK], BF16, name="et")
            nc.scalar.activation(et[:, :, 0:nq], sp[:, :, 0:nq],
                                 mybir.ActivationFunctionType.Exp)
            if j >= c * NSUB:  # diagonal block: causal triangular mask
                nc.vector.tensor_mul(et[:, :, 0:128], et[:, :, 0:128], tri2)
            ets.append((j, lv, nq, et))
        for j, lv, nq, et in ets:
            nc.tensor.matmul(cxa[:, lv:CHUNK], vaug[:, j, 2 * p, :],
                             et[:, 0, 0:nq],
                             start=(first and j == j0),
                             stop=(last and j == j1 - 1),
                             skip_group_check=True)
            nc.tensor.matmul(cxb[:, lv:CHUNK], vaug[:, j, 2 * p + 1, :],
                             et[:, 1, 0:nq],
                             start=(first and j == j0),
                             stop=(last and j == j1 - 1),
                             skip_group_check=True)

    def normalize(c, p, cxa, cxb):
        """Normalize both heads of a pair. Denominator rows (bf16) -> two
        column-tiled K=1 broadcast matmuls into one PSUM bank -> one 128-lane
        reciprocal -> two multiplies (cx PSUM x bcs SBUF, as the hardware
        partition-shift supports PSUM-side reads)."""
        cs = c * CHUNK
        rda = rc_pool.tile([1, CHUNK], BF16, tag="rda")
        nc.vector.tensor_copy(rda, cxa[DH:DH + 1, :])
        rdb = rc_pool.tile([1, CHUNK], BF16, tag="rdb")
        nc.vector.tensor_copy(rdb, cxb[DH:DH + 1, :])
        bcd = work_ps.tile([128, CHUNK], F32, tag="w", name="bcd")
        nc.tensor.matmul(bcd[0:64, :], ones64, rda, start=True, stop=True)
        nc.tensor.matmul(bcd[64:128, :], ones64, rdb, start=True, stop=True,
                         skip_group_check=True)
        bcs = rc_pool.tile([128, CHUNK], F32, tag="bcs")
        nc.vector.reciprocal_approx_fast(bcs, bcd)
        nc.vector.tensor_mul(ctxT[0:64, p, cs:cs + CHUNK],
                             cxa[0:DH, :], bcs[0:64, :])
        nc.vector.tensor_mul(ctxT[64:128, p, cs:cs + CHUNK],
                             cxb[0:DH, :], bcs[64:128, :])

    def outproj(c):
        # whole chunk into one ob tile -> ONE out-DMA (fewer queues)
        ob = ob_pool.tile([128, NSUB, D], BF16)
        for t in range(NSUB):
            gt = c * NSUB + t
            for n in range(D // 512):
                op = work_ps.tile([128, CHUNK], F32, tag="w", name="op")
                for k in range(MT):
                    nc.tensor.matmul(
                        op,
                        ctxT[:, k, gt * 128:(gt + 1) * 128],
                        wo_sb[:, k, 512 * n:512 * (n + 1)],
                        start=(k == 0), stop=(k == MT - 1))
                nc.vector.tensor_copy(ob[:, t, 512 * n:512 * (n + 1)], op)
        nc.sync.dma_start(
            out[c * CHUNK:(c + 1) * CHUNK, :].rearrange(
                "(t p) d -> p t d", p=128), ob)

    # Final chunk's out-projection is the kernel tail: split it into per-m
    # passes so the m=0 half (pair 0) runs during pair 1's attention, leaving
    # only 8 matmuls + adds + 1 DMA after the last normalize.
    def outproj_final_m0(obh):
        c = NCHUNK - 1
        for t in range(NSUB):
            gt = c * NSUB + t
            for n in range(D // 512):
                op = work_ps.tile([128, CHUNK], F32, tag="w", name="op")
                nc.tensor.matmul(op, ctxT[:, 0, gt * 128:(gt + 1) * 128],
                                 wo_sb[:, 0, 512 * n:512 * (n + 1)],
                                 start=True, stop=True)
                nc.vector.tensor_copy(obh[:, t, 512 * n:512 * (n + 1)], op)

    def outproj_final_m1(obh):
        c = NCHUNK - 1
        ob = ob_pool.tile([128, NSUB, D], BF16)
        for t in range(NSUB):
            gt = c * NSUB + t
            for n in range(D // 512):
                op = work_ps.tile([128, CHUNK], F32, tag="w", name="op")
                nc.tensor.matmul(op, ctxT[:, 1, gt * 128:(gt + 1) * 128],
                                 wo_sb[:, 1, 512 * n:512 * (n + 1)],
                                 start=True, stop=True)
                nc.vector.tensor_add(ob[:, t, 512 * n:512 * (n + 1)], op,
                                     obh[:, t, 512 * n:512 * (n + 1)])
        nc.sync.dma_start(
            out[c * CHUNK:(c + 1) * CHUNK, :].rearrange(
                "(t p) d -> p t d", p=128), ob)

    # ---- software pipeline: chunk c's attention (ScalarE-paced exp stream)
    # overlaps chunk c+1's projections (pure PE work). Without this the kernel
    # alternates PE-bound proj phases (ACT idle ~8-12us each) and ACT-bound
    # attention phases. kt2/vaug writes land in chunk c+1's column ranges,
    # disjoint from everything attention(c) reads.
    qt = proj_q(0)
    proj_k(0)
    proj_v(0)
    for c in range(NCHUNK):
        jd0, jd1 = c * NSUB, (c + 1) * NSUB
        cxa0 = cx_ps.tile([DH + 1, CHUNK], F32, tag="cxa", name="cxa0")
        cxb0 = cx_ps.tile([DH + 1, CHUNK], F32, tag="cxb", name="cxb0")
        attn(c, 0, qt, cxa0, cxb0, 0, jd0, True, False)      # off-diagonal
        attn(c, 0, qt, cxa0, cxb0, jd0, jd1, c == 0, True)   # diagonal
        normalize(c, 0, cxa0, cxb0)
        qt_next = proj_q(c + 1) if c + 1 < NCHUNK else None
        cxa1 = cx_ps.tile([DH + 1, CHUNK], F32, tag="cxa", name="cxa1")
        cxb1 = cx_ps.tile([DH + 1, CHUNK], F32, tag="cxb", name="cxb1")
        attn(c, 1, qt, cxa1, cxb1, 0, jd0, True, False)
        if c + 1 < NCHUNK:
            proj_k(c + 1)
        if c > 0:
            outproj(c - 1)
        if c == NCHUNK - 1:
            outproj_final_m0(obh)
        attn(c, 1, qt, cxa1, cxb1, jd0, jd1, c == 0, True)
        normalize(c, 1, cxa1, cxb1)
        if c + 1 < NCHUNK:
            proj_v(c + 1)
        qt = qt_next

    # dummy matmuls keep the PE HAM-warm through the final normalize chain so
    # the tail's real matmuls run at 2.4GHz instead of 1.2
    for _ in range(6):
        wp = work_ps.tile([128, CHUNK], F32, tag="w", name="wp")
        nc.tensor.matmul(wp[:, 0:256], wrm[:, 0:128], wrm[:, 0:256],
                         start=True, stop=True)
    outproj_final_m1(obh)

    for p in (cx_ps, sp_ps, work_ps, ob_pool, rc_pool, et_pool, qt_pool,
              persist, weights, consts):
        p.release()


_BUILT = None


def _build():
    global _BUILT
    if _BUILT is None:
        nc = bacc.Bacc("TRN2", target_bir_lowering=False, debug=False,
                       num_devices=NCORES)
        with tile.TileContext(nc) as tc:
            _emit(tc)
        nc.compile()
        _BUILT = nc
    return _BUILT


def _bf16(a):
    return np.ascontiguousarray(a).astype(ml_dtypes.bfloat16)


def _shards(inputs):
    x = np.asarray(inputs["x"], dtype=np.float32)
    # [p, c, k, s] chunk-major transposed x per batch
    xts = [np.ascontiguousarray(
        x[b].T.reshape(KD, 128, NCHUNK, CHUNK).transpose(1, 2, 0, 3)
    ).astype(ml_dtypes.bfloat16) for b in range(B)]
    Wq = np.asarray(inputs["Wq"], np.float32)
    Wk = np.asarray(inputs["Wk"], np.float32)
    Wv = np.asarray(inputs["Wv"], np.float32)
    Wo = np.asarray(inputs["Wo"], np.float32)
    bq_ = np.asarray(inputs["bq"], np.float32)
    bv_ = np.asarray(inputs["bv"], np.float32)
    maps = []
    for core in range(NCORES):
        b, g = core // GROUPS, core % GROUPS
        f0 = g * FEAT
        wk_f = Wk[:, f0:f0 + FEAT].reshape(KD, 128, MT, 128) \
            .transpose(1, 0, 2, 3).reshape(128, 2048)
        wv_f = Wv[:, f0:f0 + FEAT].reshape(KD, 128, FEAT) \
            .transpose(1, 0, 2).reshape(128, 2048)
        wo_f = Wo[f0:f0 + FEAT, :].reshape(MT, 128, D) \
            .transpose(1, 0, 2).reshape(128, 2048)
        br = np.empty((128, MT + FEAT), np.float32)
        br[:, 0:MT] = (bq_[f0:f0 + FEAT] * SCALE).reshape(MT, 128).T
        br[:, MT:] = bv_[f0:f0 + FEAT][None, :]
        m = {
            "xt": xts[b],
            # [p, k, m, f]
            "wq": _bf16((Wq[:, f0:f0 + FEAT] * SCALE)
                        .reshape(KD, 128, MT, 128).transpose(1, 0, 2, 3)),
            # wk | wv | wo, each flattened to [p, 2048]
            "wr": _bf16(np.stack([wk_f, wv_f, wo_f], axis=1)),
            "br": br,
        }
        maps.append(m)
    return maps


def kernel(trace=False, **inputs):
    nc = _build()
    res = run_bass_kernel_spmd(nc, _shards(inputs), core_ids=list(range(NCORES)),
                               trace=trace)
    partial = np.stack([np.asarray(r_["out"], np.float64)
                        for r_ in res.results])  # [8, S, D]
    acc = partial.reshape(B, GROUPS, S, D).sum(axis=1)
    acc += np.asarray(inputs["bo"], dtype=np.float64)
    out = acc.astype(np.float32)
    if trace:
        return out, res
    return out


# revision 17
# speedup vs baseline: 1.0137x; 1.0137x over previous
# Multi-head causal self-attention (B=2, S=2048, D=1024, H=16, Dh=64) on 8
# Trainium2 NeuronCores.
#
# Sharding: core i -> (batch b = i // 4, head-group g = i % 4). Each core
# computes attention for its batch's 4 heads (feature columns 256g:256g+256 of
# the QKV projections, rows 256g:256g+256 of Wo) and produces a partial
# out-projection [S, D]. Host sums the 4 partials per batch and adds bo.
#
# All matmul operands are bf16 (fp32 PSUM accumulation); ~4e-3 rel error,
# well under the 2e-2 gate.
#
# Schedule: ScalarE's exp stream is the attention-phase bottleneck (~1.1us
# per k-block covering both heads of a pair, (N+352)/1.2 ns) and every engine
# queue is in-order, so a dependency-stalled op head-of-line-blocks its
# engine. The emission therefore:
#   * pipelines chunk c+1's projections + chunk c-1's out-projection INTO
#     chunk c's attention emission as "filler units", pumped one unit per
#     k-block between the scores and ctx matmuls -- the PE chews filler while
#     waiting on exp;
#   * within each attention call emits all scores+exp first, then all ctx
#     (cx-bank WAR stalls after a normalize can't block the scores that feed
#     ScalarE);
#   * host pre-shapes every DMA to be fully contiguous, weights arrive in
#     consumption order, and a tiny exp at t=0 preloads the ACT table set;
#   * dummy matmuls at t=0 keep the PE busy until the first DMAs land so the
#     HAM clock-gate (4/8 -> 8/8 at ~3.4us of sustained activity) is warm
#     from the first real matmul;
#   * the final chunk's out-projection is split into per-m passes: the m=0
#     half runs during the last pair's attention, so the tail after the last
#     normalize is just 8 matmuls + adds + per-subtile DMAs.
#
# Per-core dataflow details:
#   QT = Wq_s^T xT + bq [256, S]: PSUM->SBUF move + bias ride one DVE
#   tensor_scalar_add (keeps ScalarE free for exp). KT [256, S]: K's bias is
#   dropped -- (q+bq).(k+bk) differs from (q+bq).k by a per-query-row
#   constant, which softmax cancels. Head pair p keeps head 2p on partitions
#   0:64, head 2p+1 on 64:128. V = xT^T Wv_s + bv [S, 256], augmented with a
#   ones column per head ([V_h | 1]) so the attention matmul also accumulates
#   the softmax denominator. Scores: two CONCURRENT K=64 row-tiled matmuls
#   (tile_position (0,0)/(64,0)) -> one 2-bank PSUM tile; ONE exp covers both
#   heads; scores pre-scaled by 1/sqrt(Dh) via host-side Wq scaling (small
#   enough that max-subtraction is unnecessary). Causality = skip k>q blocks
#   + one triangular mask multiply (both heads) on diagonal blocks.
#   Normalize: recip(denom) on DVE, partition-broadcast via two K=1
#   column-tiled matmuls, multiply -> ctxT bf16.

import numpy as np
import ml_dtypes

import concourse.bass as bass
import concourse.mybir as mybir
import concourse.tile as tile
from concourse import bacc
from concourse.bass_utils import run_bass_kernel_spmd
from concourse.masks import make_upper_triangular

F32 = mybir.dt.float32
BF16 = mybir.dt.bfloat16

B, S, D = 2, 2048, 1024
H, DH = 16, 64
NCORES = 8
GROUPS = 4               # head-groups (tensor parallel)
HG = H // GROUPS         # 4 heads per group
NPAIR = HG // 2          # 2 head-pairs per group
FEAT = HG * DH           # 256 features per group
SCALE = 1.0 / 8.0        # 1/sqrt(DH), folded into Wq/bq on host

CHUNK = 512              # seq chunk (PSUM bank = 512 fp32)
NSUB = CHUNK // 128      # 4 seq subtiles per chunk
NCHUNK = S // CHUNK      # 4
KD = D // 128            # 8 k-tiles over D
MT = FEAT // 128         # 2 feature M-tiles per group (m-tile == head-pair)


def _emit(tc):
    nc = tc.nc
    # host pre-shapes everything into SBUF layout -> contiguous DMAs
    xt_d = nc.dram_tensor("xt", [128, NCHUNK, KD, CHUNK], BF16,
                          kind="ExternalInput").ap()
    wq = nc.dram_tensor("wq", [128, KD, MT, 128], BF16, kind="ExternalInput").ap()
    wk = nc.dram_tensor("wk", [128, KD, MT, 128], BF16, kind="ExternalInput").ap()
    # wv | wo back to back (each a [128, 2048] bf16 view)
    wr = nc.dram_tensor("wr", [128, 2, 2048], BF16, kind="ExternalInput").ap()
    # bq (MT) | bv (HG*DH) f32
    br = nc.dram_tensor("br", [128, MT + FEAT], F32, kind="ExternalInput").ap()
    out = nc.dram_tensor("out", [S, D], BF16, kind="ExternalOutput").ap()

    consts = tc.alloc_tile_pool(name="consts", bufs=1)
    weights = tc.alloc_tile_pool(name="weights", bufs=1)
    persist = tc.alloc_tile_pool(name="persist", bufs=1)
    qt_pool = tc.alloc_tile_pool(name="qt", bufs=2)
    et_pool = tc.alloc_tile_pool(name="et", bufs=8)
    rc_pool = tc.alloc_tile_pool(name="rc", bufs=2)
    ob_pool = tc.alloc_tile_pool(name="ob", bufs=2)
    work_ps = tc.alloc_tile_pool(name="work_ps", bufs=2, space="PSUM")
    sp_ps = tc.alloc_tile_pool(name="sp_ps", bufs=2, space="PSUM")
    cx_ps = tc.alloc_tile_pool(name="cx_ps", bufs=1, space="PSUM")

    # ---- preload the ACT exp table set while DMAs are in flight
    dum = consts.tile([1, 2], F32)
    nc.gpsimd.memset(dum, 0.0)
    nc.scalar.activation(dum[0:1, 1:2], dum[0:1, 0:1],
                         mybir.ActivationFunctionType.Exp)

    # ---- x^T chunk 0 first so projections can start ASAP
    xtall = persist.tile([128, NCHUNK, KD, CHUNK], BF16)
    nc.sync.dma_start(xtall[:, 0], xt_d[:, 0])

    # ---- weights (in first-consumption order)
    wq_sb = weights.tile([128, KD, MT, 128], BF16)
    nc.sync.dma_start(wq_sb, wq)
    br_sb = weights.tile([128, MT + FEAT], F32)
    nc.sync.dma_start(br_sb, br)
    wk_sb = weights.tile([128, KD, MT, 128], BF16)
    nc.sync.dma_start(wk_sb, wk)
    wr_sb = weights.tile([128, 2, 2048], BF16)
    nc.sync.dma_start(wr_sb, wr)
    wv_sb = wr_sb[:, 0, :].rearrange("p (k f) -> p k f", k=KD)
    wo_sb = wr_sb[:, 1, :].rearrange("p (m d) -> p m d", m=MT)
    bqt = br_sb[:, 0:MT]
    bvb = br_sb[:, MT:].rearrange("p (h f) -> p h f", h=HG)

    # rest of x^T: chunk 1 alone (needed ~1 chunk in), chunks 2+3 fused
    nc.sync.dma_start(xtall[:, 1], xt_d[:, 1])
    nc.sync.dma_start(xtall[:, 2:4], xt_d[:, 2:4])

    # ---- constants
    onesf = consts.tile([128, 64], F32)
    nc.vector.memset(onesf, 1.0)
    # tri[k, q] = 1 if q >= k else 0; two copies side by side so ONE DVE op
    # masks both heads of a pair
    tri = consts.tile([128, 128], BF16)
    make_upper_triangular(nc, tri, val=1.0, diag=True)
    tri2 = consts.tile([128, 2, 128], BF16)
    nc.vector.tensor_copy(tri2[:, 0, :], tri)
    nc.vector.tensor_copy(tri2[:, 1, :], tri)
    wrm = consts.tile([128, CHUNK], BF16)
    nc.gpsimd.memset(wrm, 0.0)
    ones64 = consts.tile([1, 64], BF16)
    nc.vector.memset(ones64, 1.0)

    # ---- HAM warmup, sized to end just as the first weights/x slices arrive
    # (~13.7us; DMA-BW-bound) -- ending early lets the HAM MID window
    # re-throttle and the first chunk runs cold.
    for _ in range(26):
        wp = work_ps.tile([128, CHUNK], F32, tag="w", name="wp")
        nc.tensor.matmul(wp[:, 0:256], wrm[:, 0:128], wrm[:, 0:256],
                         start=True, stop=True)

    # ---- persistent activations
    # K^T pair-packed: pair p = heads (2p, 2p+1) on partitions 0:64 / 64:128
    kt2 = persist.tile([128, NPAIR, S], BF16)
    vaug = persist.tile([128, S // 128, HG, DH + 1], BF16)  # [V_h | 1] per head
    ctxT = persist.tile([128, MT, S], BF16)   # normalized ctx^T
    obh = persist.tile([128, NSUB, D], F32)   # final chunk's m=0 out partial
    nc.vector.tensor_copy(vaug[:, :, :, DH],
                          onesf.rearrange("p (a b) -> p a b", a=S // 128))

    # ---- filler units: single-m / single-subtile pieces of projection and
    # out-projection work, pumped into attention emission between k-blocks
    def proj_q_units(c, qt):
        def unit(m):
            def go():
                ps = work_ps.tile([128, CHUNK], F32, tag="w", name="ps")
                for k in range(KD):
                    nc.tensor.matmul(ps, wq_sb[:, k, m, :], xtall[:, c, k, :],
                                     start=(k == 0), stop=(k == KD - 1))
                nc.vector.tensor_scalar_add(qt[:, m, :], ps, bqt[:, m:m + 1])
            return go
        return [unit(m) for m in range(MT)]

    def proj_k_units(c):
        cs = c * CHUNK
        def unit(m):
            def go():
                ps = work_ps.tile([128, CHUNK], F32, tag="w", name="ps")
                for k in range(KD):
                    nc.tensor.matmul(ps, wk_sb[:, k, m, :], xtall[:, c, k, :],
                                     start=(k == 0), stop=(k == KD - 1))
                nc.vector.tensor_copy(kt2[:, m, cs:cs + CHUNK], ps)
            return go
        return [unit(m) for m in range(MT)]

    def proj_v_units(c):
        def unit(t):
            def go():
                gt = c * NSUB + t
                ps = work_ps.tile([128, CHUNK], F32, tag="w", name="ps")
                for k in range(KD):
                    nc.tensor.matmul(ps[:, 0:FEAT],
                                     xtall[:, c, k, t * 128:(t + 1) * 128],
                                     wv_sb[:, k, :],
                                     start=(k == 0), stop=(k == KD - 1))
                nc.vector.tensor_add(
                    vaug[:, gt, :, 0:DH],
                    ps[:, 0:FEAT].rearrange("p (h f) -> p h f", h=HG), bvb)
            return go
        return [unit(t) for t in range(NSUB)]

    def outproj_units(c):
        # whole chunk into one ob tile -> ONE out-DMA; tile made in unit 0
        box = []
        def unit(t):
            def go():
                if t == 0:
                    box.append(ob_pool.tile([128, NSUB, D], BF16, name="ob"))
                ob = box[0]
                gt = c * NSUB + t
                for n in range(D // 512):
                    op = work_ps.tile([128, CHUNK], F32, tag="w", name="op")
                    for k in range(MT):
                        nc.tensor.matmul(
                            op,
                            ctxT[:, k, gt * 128:(gt + 1) * 128],
                            wo_sb[:, k, 512 * n:512 * (n + 1)],
                            start=(k == 0), stop=(k == MT - 1))
                    nc.vector.tensor_copy(ob[:, t, 512 * n:512 * (n + 1)], op)
                if t == NSUB - 1:
                    nc.sync.dma_start(
                        out[c * CHUNK:(c + 1) * CHUNK, :].rearrange(
                            "(t p) d -> p t d", p=128), ob)
            return go
        return [unit(t) for t in range(NSUB)]

    def outproj_m0_units():
        # final chunk, m=0 half (pair 0) -- runs during pair 1's attention
        c = NCHUNK - 1
        def unit(t):
            def go():
                gt = c * NSUB + t
                for n in range(D // 512):
                    op = work_ps.tile([128, CHUNK], F32, tag="w", name="op")
                    nc.tensor.matmul(op, ctxT[:, 0, gt * 128:(gt + 1) * 128],
                                     wo_sb[:, 0, 512 * n:512 * (n + 1)],
                                     start=True, stop=True)
                    nc.vector.tensor_copy(obh[:, t, 512 * n:512 * (n + 1)], op)
            return go
        return [unit(t) for t in range(NSUB)]

    def outproj_m1():
        # the kernel tail: 8 matmuls + adds, per-subtile DMAs overlap the adds
        c = NCHUNK - 1
        ob = ob_pool.tile([128, NSUB, D], BF16)
        for t in range(NSUB):
            gt = c * NSUB + t
            for n in range(D // 512):
                op = work_ps.tile([128, CHUNK], F32, tag="w", name="op")
                nc.tensor.matmul(op, ctxT[:, 1, gt * 128:(gt + 1) * 128],
                                 wo_sb[:, 1, 512 * n:512 * (n + 1)],
                                 start=True, stop=True)
                nc.vector.tensor_add(ob[:, t, 512 * n:512 * (n + 1)], op,
                                     obh[:, t, 512 * n:512 * (n + 1)])
            nc.sync.dma_start(out[gt * 128:(gt + 1) * 128, :], ob[:, t, :])

    class Pump:
        def __init__(self):
            self.units = []
        def add(self, us):
            self.units.extend(us)
        def pump(self, n=1):
            for _ in range(n):
                if self.units:
                    self.units.pop(0)()
        def flush(self):
            self.pump(len(self.units))

    def attn(c, p, qt, cxa, cxb, j0, j1, first, last, pu):
        # scores+exp phase first, then ctx phase: a ctx matmul stalled on the
        # previous pair's normalize (cx-bank WAR) must not head-of-line-block
        # the scores feeding ScalarE. One filler unit after each k-block keeps
        # the PE fed while exp paces the pipeline.
        cs = c * CHUNK
        ets = []
        for j in range(j0, j1):
            lv = max(0, 128 * j - cs)   # first valid q (chunk-local)
            nq = CHUNK - lv
            sp = sp_ps.tile([128, 2, CHUNK], F32, tag="sp", name="sp")
            nc.tensor.matmul(sp[:, 0, 0:nq],
                             kt2[0:64, p, 128 * j:128 * (j + 1)],
                             qt[0:64, p, lv:CHUNK], start=True, stop=True)
            nc.tensor.matmul(sp[:, 1, 0:nq],
                             kt2[64:128, p, 128 * j:128 * (j + 1)],
                             qt[64:128, p, lv:CHUNK], start=True, stop=True)
            et = et_pool.tile([128, 2, CHUNK], BF16, name="et")
            nc.scalar.activation(et[:, :, 0:nq], sp[:, :, 0:nq],
                                 mybir.ActivationFunctionType.Exp)
            if j >= c * NSUB:  # diagonal block: causal triangular mask
                nc.vector.tensor_mul(et[:, :, 0:128], et[:, :, 0:128], tri2)
            ets.append((j, lv, nq, et))
            pu.pump()
        for j, lv, nq, et in ets:
            nc.tensor.matmul(cxa[:, lv:CHUNK], vaug[:, j, 2 * p, :],
                             et[:, 0, 0:nq],
                             start=(first and j == j0),
                             stop=(last and j == j1 - 1),
                             skip_group_check=True)
            nc.tensor.matmul(cxb[:, lv:CHUNK], vaug[:, j, 2 * p + 1, :],
                             et[:, 1, 0:nq],
                             start=(first and j == j0),
                             stop=(last and j == j1 - 1),
                             skip_group_check=True)
            pu.pump()

    def normalize(c, p, cxa, cxb):
        """Normalize both heads of a pair. Denominator rows (bf16) -> two
        column-tiled K=1 broadcast matmuls into one PSUM bank -> one 128-lane
        reciprocal -> two multiplies (cx PSUM x bcs, as the hardware
        partition-shift supports PSUM-side reads)."""
        cs = c * CHUNK
        rda = rc_pool.tile([1, CHUNK], BF16, tag="rda")
        nc.vector.tensor_copy(rda, cxa[DH:DH + 1, :])
        rdb = rc_pool.tile([1, CHUNK], BF16, tag="rdb")
        nc.vector.tensor_copy(rdb, cxb[DH:DH + 1, :])
        bcd = work_ps.tile([128, CHUNK], F32, tag="w", name="bcd")
        nc.tensor.matmul(bcd[0:64, :], ones64, rda, start=True, stop=True)
        nc.tensor.matmul(bcd[64:128, :], ones64, rdb, start=True, stop=True,
                         skip_group_check=True)
        bcs = rc_pool.tile([128, CHUNK], F32, tag="bcs")
        nc.vector.reciprocal_approx_fast(bcs, bcd)
        nc.vector.tensor_mul(ctxT[0:64, p, cs:cs + CHUNK],
                             cxa[0:DH, :], bcs[0:64, :])
        nc.vector.tensor_mul(ctxT[64:128, p, cs:cs + CHUNK],
                             cxb[0:DH, :], bcs[64:128, :])

    # ---- main pipeline: chunk 0's projections in the prologue, then chunk
    # c's attention overlapping chunk c+1's projections + chunk c-1's outproj
    qt = qt_pool.tile([128, MT, CHUNK], BF16, name="qt")
    for u in proj_q_units(0, qt):
        u()
    for u in proj_k_units(0):
        u()
    for u in proj_v_units(0):
        u()
    for c in range(NCHUNK):
        jd0, jd1 = c * NSUB, (c + 1) * NSUB
        pu = Pump()
        if c + 1 < NCHUNK:
            qt_next = qt_pool.tile([128, MT, CHUNK], BF16, name="qt")
            pu.add(proj_q_units(c + 1, qt_next))
            pu.add(proj_k_units(c + 1))
            pu.add(proj_v_units(c + 1))
        else:
            qt_next = None
        if c > 0:
            pu.add(outproj_units(c - 1))
        cxa0 = cx_ps.tile([DH + 1, CHUNK], F32, tag="cxa", name="cxa0")
        cxb0 = cx_ps.tile([DH + 1, CHUNK], F32, tag="cxb", name="cxb0")
        attn(c, 0, qt, cxa0, cxb0, 0, jd0, True, False, pu)      # off-diag
        attn(c, 0, qt, cxa0, cxb0, jd0, jd1, c == 0, True, pu)   # diagonal
        normalize(c, 0, cxa0, cxb0)
        if c == NCHUNK - 1:
            # m=0 half of the final out-projection: its ctxT half exists only
            # after normalize(c, 0), so it must not be pumped before it
            pu.add(outproj_m0_units())
        cxa1 = cx_ps.tile([DH + 1, CHUNK], F32, tag="cxa", name="cxa1")
        cxb1 = cx_ps.tile([DH + 1, CHUNK], F32, tag="cxb", name="cxb1")
        attn(c, 1, qt, cxa1, cxb1, 0, jd0, True, False, pu)
        attn(c, 1, qt, cxa1, cxb1, jd0, jd1, c == 0, True, pu)
        normalize(c, 1, cxa1, cxb1)
        pu.flush()
        qt = qt_next

    outproj_m1()

    for p in (cx_ps, sp_ps, work_ps, ob_pool, rc_pool, et_pool, qt_pool,
              persist, weights, consts):
        p.release()


_BUILT = None


def _build():
    global _BUILT
    if _BUILT is None:
        nc = bacc.Bacc("TRN2", target_bir_lowering=False, debug=False,
                       num_devices=NCORES)
        with tile.TileContext(nc) as tc:
            _emit(tc)
        nc.compile()
        _BUILT = nc
    return _BUILT


def _bf16(a):
    return np.ascontiguousarray(a).astype(ml_dtypes.bfloat16)


def _shards(inputs):
    x = np.asarray(inputs["x"], dtype=np.float32)
    # [p, c, k, s] chunk-major transposed x per batch
    xts = [np.ascontiguousarray(
        x[b].T.reshape(KD, 128, NCHUNK, CHUNK).transpose(1, 2, 0, 3)
    ).astype(ml_dtypes.bfloat16) for b in range(B)]
    Wq = np.asarray(inputs["Wq"], np.float32)
    Wk = np.asarray(inputs["Wk"], np.float32)
    Wv = np.asarray(inputs["Wv"], np.float32)
    Wo = np.asarray(inputs["Wo"], np.float32)
    bq_ = np.asarray(inputs["bq"], np.float32)
    bv_ = np.asarray(inputs["bv"], np.float32)
    maps = []
    for core in range(NCORES):
        b, g = core // GROUPS, core % GROUPS
        f0 = g * FEAT
        wv_f = Wv[:, f0:f0 + FEAT].reshape(KD, 128, FEAT) \
            .transpose(1, 0, 2).reshape(128, 2048)
        wo_f = Wo[f0:f0 + FEAT, :].reshape(MT, 128, D) \
            .transpose(1, 0, 2).reshape(128, 2048)
        br = np.empty((128, MT + FEAT), np.float32)
        br[:, 0:MT] = (bq_[f0:f0 + FEAT] * SCALE).reshape(MT, 128).T
        br[:, MT:] = bv_[f0:f0 + FEAT][None, :]
        m = {
            "xt": xts[b],
            # [p, k, m, f]
            "wq": _bf16((Wq[:, f0:f0 + FEAT] * SCALE)
                        .reshape(KD, 128, MT, 128).transpose(1, 0, 2, 3)),
            "wk": _bf16(Wk[:, f0:f0 + FEAT]
                        .reshape(KD, 128, MT, 128).transpose(1, 0, 2, 3)),
            # wv | wo, each flattened to [p, 2048]
            "wr": _bf16(np.stack([wv_f, wo_f], axis=1)),
            "br": br,
        }
        maps.append(m)
    return maps


def kernel(trace=False, **inputs):
    nc = _build()
    res = run_bass_kernel_spmd(nc, _shards(inputs), core_ids=list(range(NCORES)),
                               trace=trace)
    partial = np.stack([np.asarray(r_["out"], np.float64)
                        for r_ in res.results])  # [8, S, D]
    acc = partial.reshape(B, GROUPS, S, D).sum(axis=1)
    acc += np.asarray(inputs["bo"], dtype=np.float64)
    out = acc.astype(np.float32)
    if trace:
        return out, res
    return out


# revision 19
# speedup vs baseline: 1.0392x; 1.0251x over previous
# Multi-head causal self-attention (B=2, S=2048, D=1024, H=16, Dh=64) on 8
# Trainium2 NeuronCores.
#
# Sharding: core i -> (batch b = i // 4, head-group g = i % 4). Each core
# computes attention for its batch's 4 heads (feature columns 256g:256g+256 of
# the QKV projections, rows 256g:256g+256 of Wo) and produces a partial
# out-projection [S, D]. Host sums the 4 partials per batch and adds bo.
#
# All matmul operands are bf16 (fp32 PSUM accumulation); ~4e-3 rel error,
# well under the 2e-2 gate.
#
# Schedule: ScalarE's exp stream is the attention-phase bottleneck (~1.1us
# per k-block covering both heads of a pair, (N+352)/1.2 ns) and every engine
# queue is in-order, so a dependency-stalled op head-of-line-blocks its
# engine. The emission therefore:
#   * pipelines chunk c+1's projections + chunk c-1's out-projection INTO
#     chunk c's attention emission as "filler units", pumped one unit per
#     k-block between the scores and ctx matmuls -- the PE chews filler while
#     waiting on exp;
#   * within each attention call emits all scores+exp first, then all ctx
#     (cx-bank WAR stalls after a normalize can't block the scores that feed
#     ScalarE);
#   * host pre-shapes every DMA to be fully contiguous, weights arrive in
#     consumption order, and a tiny exp at t=0 preloads the ACT table set;
#   * dummy matmuls at t=0 keep the PE busy until the first DMAs land so the
#     HAM clock-gate (4/8 -> 8/8 at ~3.4us of sustained activity) is warm
#     from the first real matmul;
#   * the final chunk's out-projection is split into per-m passes: the m=0
#     half runs during the last pair's attention, so the tail after the last
#     normalize is just 8 matmuls + adds + per-subtile DMAs.
#
# Per-core dataflow details:
#   QT = Wq_s^T xT + bq [256, S]: PSUM->SBUF move + bias ride one DVE
#   tensor_scalar_add (keeps ScalarE free for exp). KT [256, S]: K's bias is
#   dropped -- (q+bq).(k+bk) differs from (q+bq).k by a per-query-row
#   constant, which softmax cancels. Head pair p keeps head 2p on partitions
#   0:64, head 2p+1 on 64:128. V = xT^T Wv_s + bv [S, 256], augmented with a
#   ones column per head ([V_h | 1]) so the attention matmul also accumulates
#   the softmax denominator. Scores: two CONCURRENT K=64 row-tiled matmuls
#   (tile_position (0,0)/(64,0)) -> one 2-bank PSUM tile; ONE exp covers both
#   heads; scores pre-scaled by 1/sqrt(Dh) via host-side Wq scaling (small
#   enough that max-subtraction is unnecessary). Causality = skip k>q blocks
#   + one triangular mask multiply (both heads) on diagonal blocks.
#   Normalize: recip(denom) on DVE, partition-broadcast via two K=1
#   column-tiled matmuls, multiply -> ctxT bf16.

import numpy as np
import ml_dtypes

import concourse.bass as bass
import concourse.mybir as mybir
import concourse.tile as tile
from concourse import bacc
from concourse.bass_utils import run_bass_kernel_spmd
from concourse.masks import make_upper_triangular

F32 = mybir.dt.float32
BF16 = mybir.dt.bfloat16

B, S, D = 2, 2048, 1024
H, DH = 16, 64
NCORES = 8
GROUPS = 4               # head-groups (tensor parallel)
HG = H // GROUPS         # 4 heads per group
NPAIR = HG // 2          # 2 head-pairs per group
FEAT = HG * DH           # 256 features per group
SCALE = 1.0 / 8.0        # 1/sqrt(DH), folded into Wq/bq on host

CHUNK = 512              # seq chunk (PSUM bank = 512 fp32)
NSUB = CHUNK // 128      # 4 seq subtiles per chunk
NCHUNK = S // CHUNK      # 4
KD = D // 128            # 8 k-tiles over D
MT = FEAT // 128         # 2 feature M-tiles per group (m-tile == head-pair)


def _emit(tc):
    nc = tc.nc
    # host pre-shapes everything into SBUF layout -> contiguous DMAs
    xt_d = nc.dram_tensor("xt", [128, NCHUNK, KD, CHUNK], BF16,
                          kind="ExternalInput").ap()
    wq = nc.dram_tensor("wq", [128, KD, MT, 128], BF16, kind="ExternalInput").ap()
    wk = nc.dram_tensor("wk", [128, KD, MT, 128], BF16, kind="ExternalInput").ap()
    # wv | wo back to back (each a [128, 2048] bf16 view)
    wr = nc.dram_tensor("wr", [128, 2, 2048], BF16, kind="ExternalInput").ap()
    # bq (MT) | bv (HG*DH) f32
    br = nc.dram_tensor("br", [128, MT + FEAT], F32, kind="ExternalInput").ap()
    out = nc.dram_tensor("out", [S, D], BF16, kind="ExternalOutput").ap()

    consts = tc.alloc_tile_pool(name="consts", bufs=1)
    weights = tc.alloc_tile_pool(name="weights", bufs=1)
    persist = tc.alloc_tile_pool(name="persist", bufs=1)
    qt_pool = tc.alloc_tile_pool(name="qt", bufs=2)
    et_pool = tc.alloc_tile_pool(name="et", bufs=8)
    rc_pool = tc.alloc_tile_pool(name="rc", bufs=2)
    ob_pool = tc.alloc_tile_pool(name="ob", bufs=2)
    work_ps = tc.alloc_tile_pool(name="work_ps", bufs=2, space="PSUM")
    sp_ps = tc.alloc_tile_pool(name="sp_ps", bufs=2, space="PSUM")
    cx_ps = tc.alloc_tile_pool(name="cx_ps", bufs=1, space="PSUM")

    # ---- preload the ACT exp table set while DMAs are in flight
    dum = consts.tile([1, 2], F32)
    nc.gpsimd.memset(dum, 0.0)
    nc.scalar.activation(dum[0:1, 1:2], dum[0:1, 0:1],
                         mybir.ActivationFunctionType.Exp)

    # ---- x^T chunk 0 first so projections can start ASAP
    xtall = persist.tile([128, NCHUNK, KD, CHUNK], BF16)
    nc.sync.dma_start(xtall[:, 0], xt_d[:, 0])

    # ---- weights (in first-consumption order)
    wq_sb = weights.tile([128, KD, MT, 128], BF16)
    nc.sync.dma_start(wq_sb, wq)
    br_sb = weights.tile([128, MT + FEAT], F32)
    nc.sync.dma_start(br_sb, br)
    wk_sb = weights.tile([128, KD, MT, 128], BF16)
    nc.sync.dma_start(wk_sb, wk)
    wr_sb = weights.tile([128, 2, 2048], BF16)
    nc.sync.dma_start(wr_sb, wr)
    wv_sb = wr_sb[:, 0, :].rearrange("p (k f) -> p k f", k=KD)
    wo_sb = wr_sb[:, 1, :].rearrange("p (m d) -> p m d", m=MT)
    bqt = br_sb[:, 0:MT]
    bvb = br_sb[:, MT:].rearrange("p (h f) -> p h f", h=HG)

    # rest of x^T: chunk 1 alone (needed ~1 chunk in), chunks 2+3 fused
    nc.sync.dma_start(xtall[:, 1], xt_d[:, 1])
    nc.sync.dma_start(xtall[:, 2:4], xt_d[:, 2:4])

    # ---- constants
    onesf = consts.tile([128, 64], F32)
    nc.vector.memset(onesf, 1.0)
    # tri[k, q] = 1 if q >= k else 0; two copies side by side so ONE DVE op
    # masks both heads of a pair
    tri = consts.tile([128, 128], BF16)
    make_upper_triangular(nc, tri, val=1.0, diag=True)
    tri2 = consts.tile([128, 2, 128], BF16)
    nc.vector.tensor_copy(tri2[:, 0, :], tri)
    nc.vector.tensor_copy(tri2[:, 1, :], tri)
    wrm = consts.tile([128, CHUNK], BF16)
    nc.gpsimd.memset(wrm, 0.0)
    ones64 = consts.tile([1, 64], BF16)
    nc.vector.memset(ones64, 1.0)

    # ---- HAM warmup, sized to end just as the first weights/x slices arrive
    # (~13.7us; DMA-BW-bound) -- ending early lets the HAM MID window
    # re-throttle and the first chunk runs cold.
    for _ in range(26):
        wp = work_ps.tile([128, CHUNK], F32, tag="w", name="wp")
        nc.tensor.matmul(wp[:, 0:256], wrm[:, 0:128], wrm[:, 0:256],
                         start=True, stop=True)

    # ---- persistent activations
    # K^T pair-packed: pair p = heads (2p, 2p+1) on partitions 0:64 / 64:128
    kt2 = persist.tile([128, NPAIR, S], BF16)
    vaug = persist.tile([128, S // 128, HG, DH + 1], BF16)  # [V_h | 1] per head
    ctxT = persist.tile([128, MT, S], BF16)   # normalized ctx^T
    obh = persist.tile([128, NSUB, D], F32)   # final chunk's m=0 out partial
    nc.vector.tensor_copy(vaug[:, :, :, DH],
                          onesf.rearrange("p (a b) -> p a b", a=S // 128))

    # ---- filler units: single-m / single-subtile pieces of projection and
    # out-projection work, pumped into attention emission between k-blocks
    def proj_q_units(c, qt):
        def unit(m):
            def go():
                ps = work_ps.tile([128, CHUNK], F32, tag="w", name="ps")
                for k in range(KD):
                    nc.tensor.matmul(ps, wq_sb[:, k, m, :], xtall[:, c, k, :],
                                     start=(k == 0), stop=(k == KD - 1))
                nc.vector.tensor_scalar_add(qt[:, m, :], ps, bqt[:, m:m + 1])
            return go
        return [unit(m) for m in range(MT)]

    def proj_k_units(c):
        cs = c * CHUNK
        def unit(m):
            def go():
                ps = work_ps.tile([128, CHUNK], F32, tag="w", name="ps")
                for k in range(KD):
                    nc.tensor.matmul(ps, wk_sb[:, k, m, :], xtall[:, c, k, :],
                                     start=(k == 0), stop=(k == KD - 1))
                nc.vector.tensor_copy(kt2[:, m, cs:cs + CHUNK], ps)
            return go
        return [unit(m) for m in range(MT)]

    def proj_v_units(c):
        def unit(t):
            def go():
                gt = c * NSUB + t
                ps = work_ps.tile([128, CHUNK], F32, tag="w", name="ps")
                for k in range(KD):
                    nc.tensor.matmul(ps[:, 0:FEAT],
                                     xtall[:, c, k, t * 128:(t + 1) * 128],
                                     wv_sb[:, k, :],
                                     start=(k == 0), stop=(k == KD - 1))
                nc.vector.tensor_add(
                    vaug[:, gt, :, 0:DH],
                    ps[:, 0:FEAT].rearrange("p (h f) -> p h f", h=HG), bvb)
            return go
        return [unit(t) for t in range(NSUB)]

    def outproj_units(c):
        # whole chunk into one ob tile -> ONE out-DMA; tile made in unit 0
        box = []
        def unit(t):
            def go():
                if t == 0:
                    box.append(ob_pool.tile([128, NSUB, D], BF16, name="ob"))
                ob = box[0]
                gt = c * NSUB + t
                for n in range(D // 512):
                    op = work_ps.tile([128, CHUNK], F32, tag="w", name="op")
                    for k in range(MT):
                        nc.tensor.matmul(
                            op,
                            ctxT[:, k, gt * 128:(gt + 1) * 128],
                            wo_sb[:, k, 512 * n:512 * (n + 1)],
                            start=(k == 0), stop=(k == MT - 1))
                    nc.vector.tensor_copy(ob[:, t, 512 * n:512 * (n + 1)], op)
                if t == NSUB - 1:
                    nc.sync.dma_start(
                        out[c * CHUNK:(c + 1) * CHUNK, :].rearrange(
                            "(t p) d -> p t d", p=128), ob)
            return go
        return [unit(t) for t in range(NSUB)]

    def outproj_m0_units():
        # final chunk, m=0 half (pair 0) -- runs during pair 1's attention
        c = NCHUNK - 1
        def unit(t):
            def go():
                gt = c * NSUB + t
                for n in range(D // 512):
                    op = work_ps.tile([128, CHUNK], F32, tag="w", name="op")
                    nc.tensor.matmul(op, ctxT[:, 0, gt * 128:(gt + 1) * 128],
                                     wo_sb[:, 0, 512 * n:512 * (n + 1)],
                                     start=True, stop=True)
                    nc.vector.tensor_copy(obh[:, t, 512 * n:512 * (n + 1)], op)
            return go
        return [unit(t) for t in range(NSUB)]

    def outproj_m1():
        # the kernel tail: 8 matmuls + adds, per-subtile DMAs overlap the adds
        c = NCHUNK - 1
        ob = ob_pool.tile([128, NSUB, D], BF16)
        for t in range(NSUB):
            gt = c * NSUB + t
            for n in range(D // 512):
                op = work_ps.tile([128, CHUNK], F32, tag="w", name="op")
                nc.tensor.matmul(op, ctxT[:, 1, gt * 128:(gt + 1) * 128],
                                 wo_sb[:, 1, 512 * n:512 * (n + 1)],
                                 start=True, stop=True)
                nc.vector.tensor_add(ob[:, t, 512 * n:512 * (n + 1)], op,
                                     obh[:, t, 512 * n:512 * (n + 1)])
            nc.sync.dma_start(out[gt * 128:(gt + 1) * 128, :], ob[:, t, :])

    def attn(c, p, qt, cxa, cxb, j0, j1, first, last):
        # scores+exp phase first, then ctx phase: a ctx matmul stalled on the
        # previous pair's normalize (cx-bank WAR) must not head-of-line-block
        # the scores feeding ScalarE.
        cs = c * CHUNK
        ets = []
        for j in range(j0, j1):
            lv = max(0, 128 * j - cs)   # first valid q (chunk-local)
            nq = CHUNK - lv
            sp = sp_ps.tile([128, 2, CHUNK], F32, tag="sp", name="sp")
            nc.tensor.matmul(sp[:, 0, 0:nq],
                             kt2[0:64, p, 128 * j:128 * (j + 1)],
                             qt[0:64, p, lv:CHUNK], start=True, stop=True)
            nc.tensor.matmul(sp[:, 1, 0:nq],
                             kt2[64:128, p, 128 * j:128 * (j + 1)],
                             qt[64:128, p, lv:CHUNK], start=True, stop=True)
            et = et_pool.tile([128, 2, CHUNK], BF16, name="et")
            nc.scalar.activation(et[:, :, 0:nq], sp[:, :, 0:nq],
                                 mybir.ActivationFunctionType.Exp)
            if j >= c * NSUB:  # diagonal block: causal triangular mask
                nc.vector.tensor_mul(et[:, :, 0:128], et[:, :, 0:128], tri2)
            ets.append((j, lv, nq, et))
        for j, lv, nq, et in ets:
            nc.tensor.matmul(cxa[:, lv:CHUNK], vaug[:, j, 2 * p, :],
                             et[:, 0, 0:nq],
                             start=(first and j == j0),
                             stop=(last and j == j1 - 1),
                             skip_group_check=True)
            nc.tensor.matmul(cxb[:, lv:CHUNK], vaug[:, j, 2 * p + 1, :],
                             et[:, 1, 0:nq],
                             start=(first and j == j0),
                             stop=(last and j == j1 - 1),
                             skip_group_check=True)

    def normalize(c, p, cxa, cxb):
        """Normalize both heads of a pair. Denominator rows (bf16) -> two
        column-tiled K=1 broadcast matmuls into one PSUM bank -> one 128-lane
        reciprocal -> two multiplies (cx PSUM x bcs, as the hardware
        partition-shift supports PSUM-side reads)."""
        cs = c * CHUNK
        rda = rc_pool.tile([1, CHUNK], BF16, tag="rda")
        nc.vector.tensor_copy(rda, cxa[DH:DH + 1, :])
        rdb = rc_pool.tile([1, CHUNK], BF16, tag="rdb")
        nc.vector.tensor_copy(rdb, cxb[DH:DH + 1, :])
        bcd = work_ps.tile([128, CHUNK], F32, tag="w", name="bcd")
        nc.tensor.matmul(bcd[0:64, :], ones64, rda, start=True, stop=True)
        nc.tensor.matmul(bcd[64:128, :], ones64, rdb, start=True, stop=True,
                         skip_group_check=True)
        bcs = rc_pool.tile([128, CHUNK], F32, tag="bcs")
        nc.vector.reciprocal_approx_fast(bcs, bcd)
        nc.vector.tensor_mul(ctxT[0:64, p, cs:cs + CHUNK],
                             cxa[0:DH, :], bcs[0:64, :])
        nc.vector.tensor_mul(ctxT[64:128, p, cs:cs + CHUNK],
                             cxb[0:DH, :], bcs[64:128, :])

    # ---- main pipeline: chunk 0's projections in the prologue, then chunk
    # c's attention overlapping chunk c+1's projections + chunk c-1's outproj
    qt = qt_pool.tile([128, MT, CHUNK], BF16, name="qt")
    for u in proj_q_units(0, qt):
        u()
    for u in proj_k_units(0):
        u()
    for u in proj_v_units(0):
        u()
    for c in range(NCHUNK):
        jd0, jd1 = c * NSUB, (c + 1) * NSUB
        cxa0 = cx_ps.tile([DH + 1, CHUNK], F32, tag="cxa", name="cxa0")
        cxb0 = cx_ps.tile([DH + 1, CHUNK], F32, tag="cxb", name="cxb0")
        attn(c, 0, qt, cxa0, cxb0, 0, jd0, True, False)      # off-diagonal
        attn(c, 0, qt, cxa0, cxb0, jd0, jd1, c == 0, True)   # diagonal
        normalize(c, 0, cxa0, cxb0)
        if c + 1 < NCHUNK:
            qt_next = qt_pool.tile([128, MT, CHUNK], BF16, name="qt")
            for u in proj_q_units(c + 1, qt_next):
                u()
        else:
            qt_next = None
        cxa1 = cx_ps.tile([DH + 1, CHUNK], F32, tag="cxa", name="cxa1")
        cxb1 = cx_ps.tile([DH + 1, CHUNK], F32, tag="cxb", name="cxb1")
        attn(c, 1, qt, cxa1, cxb1, 0, jd0, True, False)
        if c + 1 < NCHUNK:
            for u in proj_k_units(c + 1):
                u()
        if c > 0:
            for u in outproj_units(c - 1):
                u()
        if c == NCHUNK - 1:
            for u in outproj_m0_units():
                u()
        attn(c, 1, qt, cxa1, cxb1, jd0, jd1, c == 0, True)
        normalize(c, 1, cxa1, cxb1)
        if c + 1 < NCHUNK:
            for u in proj_v_units(c + 1):
                u()
        qt = qt_next

    outproj_m1()

    for p in (cx_ps, sp_ps, work_ps, ob_pool, rc_pool, et_pool, qt_pool,
              persist, weights, consts):
        p.release()


_BUILT = None


def _build():
    global _BUILT
    if _BUILT is None:
        nc = bacc.Bacc("TRN2", target_bir_lowering=False, debug=False,
                       num_devices=NCORES)
        with tile.TileContext(nc) as tc:
            _emit(tc)
        nc.compile()
        _BUILT = nc
    return _BUILT


def _bf16(a):
    return np.ascontiguousarray(a).astype(ml_dtypes.bfloat16)


def _shards(inputs):
    x = np.asarray(inputs["x"], dtype=np.float32)
    # [p, c, k, s] chunk-major transposed x per batch
    xts = [np.ascontiguousarray(
        x[b].T.reshape(KD, 128, NCHUNK, CHUNK).transpose(1, 2, 0, 3)
    ).astype(ml_dtypes.bfloat16) for b in range(B)]
    Wq = np.asarray(inputs["Wq"], np.float32)
    Wk = np.asarray(inputs["Wk"], np.float32)
    Wv = np.asarray(inputs["Wv"], np.float32)
    Wo = np.asarray(inputs["Wo"], np.float32)
    bq_ = np.asarray(inputs["bq"], np.float32)
    bv_ = np.asarray(inputs["bv"], np.float32)
    maps = []
    for core in range(NCORES):
        b, g = core // GROUPS, core % GROUPS
        f0 = g * FEAT
        wv_f = Wv[:, f0:f0 + FEAT].reshape(KD, 128, FEAT) \
            .transpose(1, 0, 2).reshape(128, 2048)
        wo_f = Wo[f0:f0 + FEAT, :].reshape(MT, 128, D) \
            .transpose(1, 0, 2).reshape(128, 2048)
        br = np.empty((128, MT + FEAT), np.float32)
        br[:, 0:MT] = (bq_[f0:f0 + FEAT] * SCALE).reshape(MT, 128).T
        br[:, MT:] = bv_[f0:f0 + FEAT][None, :]
        m = {
            "xt": xts[b],
            # [p, k, m, f]
            "wq": _bf16((Wq[:, f0:f0 + FEAT] * SCALE)
                        .reshape(KD, 128, MT, 128).transpose(1, 0, 2, 3)),
            "wk": _bf16(Wk[:, f0:f0 + FEAT]
                        .reshape(KD, 128, MT, 128).transpose(1, 0, 2, 3)),
            # wv | wo, each flattened to [p, 2048]
            "wr": _bf16(np.stack([wv_f, wo_f], axis=1)),
            "br": br,
        }
        maps.append(m)
    return maps


def kernel(trace=False, **inputs):
    nc = _build()
    res = run_bass_kernel_spmd(nc, _shards(inputs), core_ids=list(range(NCORES)),
                               trace=trace)
    partial = np.stack([np.asarray(r_["out"], np.float64)
                        for r_ in res.results])  # [8, S, D]
    acc = partial.reshape(B, GROUPS, S, D).sum(axis=1)
    acc += np.asarray(inputs["bo"], dtype=np.float64)
    out = acc.astype(np.float32)
    if trace:
        return out, res
    return out
